# revision 2
# baseline (speedup 1.0000x reference)
"""Trainium2 Bass kernel for nn_ProteinGAT (2-layer GATConv + global mean pool).

v5: FP8 compact table + single spanning AllGather per layer.
  - Nodes dst-sharded (npc=N/8 per core). Per layer a compact per-node row
    [hs fp8 x64 | 1.0 fp8 | pad | a_src bf16 (2B)] = 68B is packed into
    d_slice, AllGathered once (N x 68B = 3.4MB -> ~100us), then expanded by
    16 per-owner DMAs into two 256B-pitch bucket tables (dma_gather needs
    256B row pitch; two buckets keep gather indices in int16).
  - Collectives hard-block their engine queue in the cost model, so the AG
    sits on the Pool queue where nothing can overlap it; minimizing the
    makespan = minimizing sum(gather time) + sum(collective time). FP8
    halves collective bytes; hs noise (~3.6%/elem) averages out over ~24
    edges per node (verified ~2e-3 final rel err, gate is 2e-2).
  - Edge phase w-major: per (w,b) a PSUM window accumulates S'/denom via
    p-scaled one-hot matmuls with stationary = gathered rows upconverted
    to bf16 on the ACT engine; the epilogue sums both buckets' windows,
    h = relu(S')/denom, then inlines next-layer pack + adst (layer 0) or
    pooling (layer 1) for that window.
  - exp(lrelu(y)) == max(exp(y), exp(0.2*y)): same Exp table, no
    activation-table reloads. a_edge is folded into the one-hot mask
    host-side (eamask = a_edge | -1000).
Accepted deviations: isolated nodes give h=0 (gat_bias==0); softmax without
max subtraction (logits O(0.1)).
"""

import numpy as np
import ml_dtypes

import concourse.bass as bass
import concourse.bacc as bacc
import concourse.mybir as mybir
import concourse.tile as tile
from concourse.bass_utils import run_bass_kernel_spmd

F32 = mybir.dt.float32
BF16 = mybir.dt.bfloat16
FP8 = mybir.dt.float8e4
I16 = mybir.dt.int16
I32 = mybir.dt.int32
AF = mybir.ActivationFunctionType
OP = mybir.AluOpType

TROW = 128          # expanded table row pitch in bf16 elems (256B); the
                    # CONTENT is fp8-packed (gather descriptors scale with
                    # element count, so gather as bf16: 1 desc/row not 2)
TROWC = 68          # compact table row width in bytes (fp8 elems)
HS = 64
NSTA = 65           # stationary cols: 64 hs + one
COL_ONE = 64
ROW_DEN = 64
WIN = 512
SUB = 16
BMAX = 24
GCALL = 8
ALPHA = 0.2
EPS = 1e-16


class Cfg:
    def __init__(self, N, E, G, n_cores, F_IN=128, chunk_wins=(6,)):
        self.N, self.E, self.G, self.n_cores, self.F_IN = N, E, G, n_cores, F_IN
        assert N % n_cores == 0
        self.npc = N // n_cores
        self.nwin = -(-self.npc // WIN)
        self.npad = self.nwin * WIN
        self.ntile = -(-self.npc // 128)
        self.spw = WIN // SUB
        # bucket boundaries in windows: bucket b = windows [wb[b], wb[b+1])
        self.wb = (0,) + tuple(chunk_wins) + (self.nwin,)
        self.k = len(self.wb) - 1
        assert all(self.wb[i] < self.wb[i + 1] for i in range(self.k))
        self.c_lo = [min(w * WIN, self.npc) for w in self.wb]
        self.c_sz = [self.c_lo[c + 1] - self.c_lo[c] for c in range(self.k)]
        self.rows_c = [n_cores * s for s in self.c_sz]
        assert all(r <= 32768 for r in self.rows_c)


# ---------------------------------------------------------------------------
# host preprocessing
# ---------------------------------------------------------------------------

def _plan_core(src, dloc, cfg):
    """groups[(w,b,s)] = local edge idx; row = per-edge bucket-table row."""
    owner = src // cfg.npc
    loc = src - owner * cfg.npc
    b_of = np.searchsorted(np.array(cfg.c_lo[1:-1]), loc, side="right") \
        if cfg.k > 1 else np.zeros(len(src), np.int64)
    lo_arr = np.array([cfg.c_lo[c] for c in range(cfg.k)])
    sz_arr = np.array(cfg.c_sz)
    row = owner * sz_arr[b_of] + (loc - lo_arr[b_of])
    groups = {}
    for b in range(cfg.k):
        sel = np.nonzero(b_of == b)[0]
        s_sub = dloc[sel] // SUB
        order = np.argsort(s_sub, kind="stable")
        sel, s_sub = sel[order], s_sub[order]
        nsub = cfg.npad // SUB
        lo = np.searchsorted(s_sub, np.arange(nsub))
        hi = np.append(lo[1:], len(sel))
        for s in range(nsub):
            if hi[s] > lo[s]:
                groups[(s // cfg.spw, b, s)] = sel[lo[s]:hi[s]]
    return groups, row


def _structure(cfg, all_groups):
    """Static structure shared by all cores (window-major, bucket inner).
    act[t]=1 marks the last tile of (w, last bucket) -> epilogue."""
    T = np.zeros((cfg.nwin, cfg.k), np.int64)
    for groups in all_groups:
        for (w, b, s), ed in groups.items():
            T[w, b] = max(T[w, b], -(-len(ed) // 128))
    T = np.maximum(T, 1)
    tiles, runs = [], []
    for w in range(cfg.nwin):
        for b in range(cfg.k):
            t_per = int(T[w, b])
            ks_max = max(1, BMAX // t_per)
            s = 0
            while s < cfg.spw:
                ks = min(ks_max, cfg.spw - s)
                lo = len(tiles)
                for q in range(ks):
                    tiles += [(w, b, w * cfg.spw + s + q)] * t_per
                runs.append((w, b, lo, ks * t_per, s, ks, t_per))
                s += ks
    last_w, last_wb = {}, {}
    for t, (w, b, s) in enumerate(tiles):
        last_w[w] = t
        last_wb[(w, b)] = t
    act = [0] * len(tiles)
    for w, t in last_w.items():
        act[t] = 1
    stop = [False] * len(tiles)
    for (w, b), t in last_wb.items():
        stop[t] = True
    return T, tiles, runs, act, stop


def preprocess(inputs, cfg):
    x = np.asarray(inputs["x"], np.float32)
    ea_v = np.asarray(inputs["edge_attr"], np.float32)
    ei = np.asarray(inputs["edge_index"]).astype(np.int64)
    batch = np.asarray(inputs["batch"]).astype(np.int64)
    lin_W = np.asarray(inputs["lin_W"], np.float32)
    att_src = np.asarray(inputs["att_src"], np.float32)
    att_dst = np.asarray(inputs["att_dst"], np.float32)
    lin_edge_W = np.asarray(inputs["lin_edge_W"], np.float32)
    att_edge = np.asarray(inputs["att_edge"], np.float32)
    gat_bias = np.asarray(inputs["gat_bias"], np.float32)
    W_embed = np.asarray(inputs["W_embed"], np.float32)
    b_embed = np.asarray(inputs["b_embed"], np.float32)

    c = [float(lin_edge_W[l, 0] @ att_edge[l]) for l in range(2)]
    A0 = W_embed @ lin_W[0]
    W0_ext = np.concatenate([A0, (A0 @ att_src[0])[:, None]], 1)
    W0_dst = (A0 @ att_dst[0])[:, None]
    b0v = b_embed @ lin_W[0]
    b0_ext = np.concatenate([b0v + gat_bias[0], [b0v @ att_src[0]]])
    b0_dst = float(b0v @ att_dst[0])
    W1_ext = np.concatenate([lin_W[1], (lin_W[1] @ att_src[1])[:, None]], 1)
    W1_dst = (lin_W[1] @ att_dst[1])[:, None]
    b1_ext = np.concatenate([gat_bias[1], [0.0]])

    src, dst = ei[0], ei[1]
    per_core = []
    for cid in range(cfg.n_cores):
        n0 = cid * cfg.npc
        m = (dst >= n0) & (dst < n0 + cfg.npc)
        src_c, dloc_c = src[m], dst[m] - n0
        groups, row = _plan_core(src_c, dloc_c, cfg)
        per_core.append((row, dloc_c, np.nonzero(m)[0], groups))
    T, tiles, runs, act, stop = _structure(cfg, [p[3] for p in per_core])
    NT = len(tiles)

    in_maps = []
    for cid in range(cfg.n_cores):
        row_c, dloc_c, orig, groups = per_core[cid]
        gidx = np.zeros((128, NT * 8), np.int16)
        eamask = np.full((2, 128, NT, SUB), -1000.0, np.float32)
        cursor = {}
        for t, (w, b, s) in enumerate(tiles):
            kk = cursor.get((w, b, s), 0)
            cursor[(w, b, s)] = kk + 1
            ed = groups.get((w, b, s), np.zeros(0, np.int64))
            ed = ed[kk * 128:(kk + 1) * 128]
            n = len(ed)
            if n:
                gf = np.zeros(128, np.int16)
                gf[:n] = row_c[ed].astype(np.int16)
                gidx[:, t * 8:(t + 1) * 8] = np.tile(gf.reshape(8, 16).T, (8, 1))
                cols = dloc_c[ed] - s * SUB
                for l in range(2):
                    eamask[l, np.arange(n), t, cols] = ea_v[orig[ed]] * c[l]
        n0 = cid * cfg.npc
        xs = np.zeros((cfg.F_IN, cfg.npad), np.float32)
        xs[:, :cfg.npc] = x[n0:n0 + cfg.npc].T
        ind = np.zeros((128, cfg.ntile, cfg.G), np.float32)
        bloc = batch[n0:n0 + cfg.npc]
        for t in range(cfg.ntile):
            rows = bloc[t * 128:(t + 1) * 128]
            ind[np.arange(len(rows)), t, rows] = 1.0
        in_maps.append({
            "xT": xs.astype(ml_dtypes.bfloat16),
            "gidx": gidx,
            "mask0": eamask[0].reshape(128, NT * SUB).astype(ml_dtypes.bfloat16),
            "mask1": eamask[1].reshape(128, NT * SUB).astype(ml_dtypes.bfloat16),
            "W0_ext": W0_ext.astype(ml_dtypes.bfloat16),
            "W0_dst": W0_dst.astype(ml_dtypes.bfloat16),
            "W1_ext": W1_ext.astype(ml_dtypes.bfloat16),
            "W1_dst": W1_dst.astype(ml_dtypes.bfloat16),
            "b0_ext": np.broadcast_to(b0_ext, (128, 65)).astype(np.float32).copy(),
            "b1_ext": np.broadcast_to(b1_ext, (128, 65)).astype(np.float32).copy(),
            "ind": ind.astype(ml_dtypes.bfloat16),
        })
    st = dict(T=T, tiles=tiles, runs=runs, act=act, stop=stop,
              NT=NT, b0_dst=b0_dst)
    return in_maps, st


# ---------------------------------------------------------------------------
# device program
# ---------------------------------------------------------------------------

def build_program(cfg, st):
    NT = st["NT"]
    tiles, runs, act, stop = (st["tiles"], st["runs"], st["act"],
                              st["stop"])
    F_IN = cfg.F_IN

    nc = bacc.Bacc("TRN2", target_bir_lowering=False, debug=False,
                   num_devices=cfg.n_cores)
    dt = nc.dram_tensor
    i_xT = dt("xT", [F_IN, cfg.npad], BF16, kind="ExternalInput")
    i_gidx = dt("gidx", [128, NT * 8], I16, kind="ExternalInput")
    i_mask = [dt("mask0", [128, NT * SUB], BF16, kind="ExternalInput"),
              dt("mask1", [128, NT * SUB], BF16, kind="ExternalInput")]
    i_W_ext = [dt("W0_ext", [F_IN, 65], BF16, kind="ExternalInput"),
               dt("W1_ext", [HS, 65], BF16, kind="ExternalInput")]
    i_W_dst = [dt("W0_dst", [F_IN, 1], BF16, kind="ExternalInput"),
               dt("W1_dst", [HS, 1], BF16, kind="ExternalInput")]
    i_b_ext = [dt("b0_ext", [128, 65], F32, kind="ExternalInput"),
               dt("b1_ext", [128, 65], F32, kind="ExternalInput")]
    i_ind = dt("ind", [128, cfg.ntile, cfg.G], BF16, kind="ExternalInput")
    o_gsum = dt("gsum", [cfg.G, HS], F32, kind="ExternalOutput")

    d_slice = dt("dslice", [cfg.npc, TROWC], FP8)
    d_tabc = [dt(f"tabc{l}", [cfg.N, TROWC], FP8, addr_space="Shared")
              for l in range(2)]
    d_tab = [[dt(f"tab{l}_{b}", [cfg.rows_c[b], TROW], BF16)
              for b in range(cfg.k)] for l in range(2)]

    with tile.TileContext(nc) as tc:
      with tc.tile_pool(name="res", bufs=1) as res, \
           tc.tile_pool(name="chunkp", bufs=4) as chunkp, \
           tc.tile_pool(name="chbp", bufs=4) as chbp, \
           tc.tile_pool(name="gridp", bufs=4) as gridp, \
           tc.tile_pool(name="ohp", bufs=4) as ohp, \
           tc.tile_pool(name="winp", bufs=4, space="PSUM") as winp, \
           tc.tile_pool(name="psmall", bufs=3, space="PSUM") as psmall, \
           tc.tile_pool(name="gsp", bufs=1, space="PSUM") as gsp, \
           tc.tile_pool(name="packp", bufs=4) as packp, \
           tc.tile_pool(name="evp", bufs=2) as evp:

        # ---- residents & constants ----
        xT_sb = res.tile([F_IN, cfg.npad], BF16)
        nc.sync.dma_start(out=xT_sb[:, :], in_=i_xT[:, :])
        W_ext_sb, W_dst_sb, b_ext_sb = [], [], []
        for l in range(2):
            kdim = F_IN if l == 0 else HS
            wx = res.tile([kdim, 65], BF16, name=f"wext{l}")
            nc.sync.dma_start(out=wx[:, :], in_=i_W_ext[l][:, :])
            W_ext_sb.append(wx)
            wd = res.tile([kdim, 1], BF16, name=f"wdst{l}")
            nc.sync.dma_start(out=wd[:, :], in_=i_W_dst[l][:, :])
            W_dst_sb.append(wd)
            bx = res.tile([128, 65], F32, name=f"bext{l}")
            nc.sync.dma_start(out=bx[:, :], in_=i_b_ext[l][:, :])
            b_ext_sb.append(bx)
        ind_sb = res.tile([128, cfg.ntile, cfg.G], BF16)
        nc.sync.dma_start(out=ind_sb[:, :, :], in_=i_ind[:, :, :])

        zsta = res.tile([128, NSTA], BF16)
        nc.vector.memset(zsta[:, :], 0.0)
        zmov = res.tile([128, WIN], BF16)
        nc.vector.memset(zmov[:, :], 0.0)
        ones1 = res.tile([1, 128], BF16)
        nc.vector.memset(ones1[:, :], 1.0)
        one11 = res.tile([1, 1], F32)
        nc.vector.memset(one11[:, :], 1.0)
        idn_i = res.tile([HS, HS], I32)
        nc.gpsimd.iota(idn_i[:, :], pattern=[[1, HS]], base=0,
                       channel_multiplier=-1)
        idn = res.tile([HS, HS], BF16)
        nc.vector.tensor_scalar(idn[:, :], idn_i[:, :], 0.0, None,
                                op0=OP.is_equal)

        adst_sb = [res.tile([128, cfg.npad], BF16, name=f"adst{l}")
                   for l in range(2)]
        rrow_sb = res.tile([1, cfg.npad], F32)
        rcol_sb = res.tile([128, cfg.ntile], F32)
        hT_sb = res.tile([HS, cfg.npad], BF16)   # relu'd, UNSCALED h^T

        def pack_tile(l, t):
            hprev = xT_sb if l == 0 else hT_sb
            pp = psmall.tile([128, 65], F32, name="pp", tag="ps")
            nc.tensor.matmul(pp[:, :], hprev[:, t * 128:(t + 1) * 128],
                             W_ext_sb[l][:, :], start=True, stop=True)
            ts = packp.tile([128, TROWC], FP8, name="tsl", tag="tsl")
            a_f = packp.tile([128, 1], F32, name="a_f", tag="a_f")
            if l == 0:
                nc.vector.tensor_tensor(ts[:, 0:64], pp[:, 0:64],
                                        b_ext_sb[l][:, 0:64], op=OP.add)
                nc.vector.tensor_tensor(a_f[:, :], pp[:, 64:65],
                                        b_ext_sb[l][:, 64:65], op=OP.add)
            else:
                sc = packp.tile([128, 65], F32, name="sc", tag="sc")
                nc.vector.tensor_scalar(sc[:, :], pp[:, :],
                                        rcol_sb[:, t:t + 1], None,
                                        op0=OP.mult)
                nc.vector.tensor_tensor(ts[:, 0:64], sc[:, 0:64],
                                        b_ext_sb[l][:, 0:64], op=OP.add)
                nc.vector.tensor_tensor(a_f[:, :], sc[:, 64:65],
                                        b_ext_sb[l][:, 64:65], op=OP.add)
            nc.vector.memset(ts[:, COL_ONE:COL_ONE + 2], 1.0)
            nc.vector.tensor_copy(ts[:, 66:68].bitcast(BF16), a_f[:, :])
            n_r = min(128, cfg.npc - t * 128)
            nc.sync.dma_start(out=d_slice[t * 128:t * 128 + n_r, :],
                              in_=ts[0:n_r, :])

        # scheduler-only estimates of when each layer's collective lands;
        # expand DMAs have no tile deps, so without wait hints the tile
        # scheduler hoists them to the ACT-queue front where their runtime
        # sem-wait head-of-line-blocks the pass's activations.
        agns = 15000.0 + cfg.N * TROWC / 40.0
        ag_end = [35000.0 + agns, 0.0]
        pass_ns = (NT / GCALL) * 920.0
        ag_end[1] = ag_end[0] + 10000.0 + pass_ns + 10000.0 + agns

        def fire_ag(l):
            nc.gpsimd.collective_compute(
                "AllGather", OP.bypass,
                replica_groups=[list(range(cfg.n_cores))],
                ins=[d_slice.ap().opt()],
                outs=[d_tabc[l].ap().opt()],
            )

        def expand(l):
            """compact [N,68B] fp8 -> two 256B-pitch bucket tables;
            one 2-level DMA per (owner, bucket)."""
            with tc.tile_wait_until(ag_end[l] / 1e6):
                for b in range(cfg.k):
                    lo, sz = cfg.c_lo[b], cfg.c_sz[b]
                    for o in range(cfg.n_cores):
                        nc.scalar.dma_start(
                            out=d_tab[l][b][o * sz:(o + 1) * sz,
                                            0:TROWC // 2],
                            in_=d_tabc[l][o * cfg.npc + lo:
                                          o * cfg.npc + lo + sz,
                                          :].bitcast(BF16))

        def adst_win(l, w):
            hprev = xT_sb if l == 0 else hT_sb
            pa = psmall.tile([1, WIN], F32, name="pa", tag="ps")
            nc.tensor.matmul(pa[:, :], W_dst_sb[l][:, :],
                             hprev[:, w * WIN:(w + 1) * WIN],
                             start=True, stop=True)
            ab = evp.tile([1, WIN], BF16, name="ab", tag="ab")
            if l == 0:
                nc.vector.tensor_scalar(ab[:, :], pa[:, :],
                                        float(st["b0_dst"]), None, op0=OP.add)
            else:
                nc.vector.tensor_tensor(ab[:, :], pa[:, :],
                                        rrow_sb[:, w * WIN:(w + 1) * WIN],
                                        op=OP.mult)
            pb = psmall.tile([128, WIN], F32, name="pb", tag="ps")
            nc.tensor.matmul(pb[:, :], ones1[:, :], ab[:, :],
                             start=True, stop=True)
            nc.vector.tensor_copy(adst_sb[l][:, w * WIN:(w + 1) * WIN],
                                  pb[:, :])

        gs_tile = [None]

        def epilogue_final(l, w, wps):
            """h = relu(sum_b S')/denom for window w; write hT, rrow, rcol;
            then inline next-layer pack/adst (l==0) or pooling (l==1)."""
            ws = slice(w * WIN, (w + 1) * WIN)
            rr = rrow_sb[:, ws]
            if len(wps) > 1:
                # DVE can read only one PSUM operand per op: stage via SBUF
                nc.vector.tensor_scalar(rr, wps[0][ROW_DEN:ROW_DEN + 1, :],
                                        EPS, None, op0=OP.add)
                nc.vector.tensor_tensor(rr, rr,
                                        wps[1][ROW_DEN:ROW_DEN + 1, :],
                                        op=OP.add)
                nc.vector.tensor_copy(hT_sb[:, ws], wps[0][0:HS, :])
                nc.vector.tensor_tensor(hT_sb[:, ws], hT_sb[:, ws],
                                        wps[1][0:HS, :], op=OP.add)
                nc.vector.tensor_scalar(hT_sb[:, ws], hT_sb[:, ws], 0.0,
                                        None, op0=OP.max)
            else:
                nc.vector.tensor_scalar(rr, wps[0][ROW_DEN:ROW_DEN + 1, :],
                                        EPS, None, op0=OP.add)
                nc.vector.tensor_scalar(hT_sb[:, ws], wps[0][0:HS, :], 0.0,
                                        None, op0=OP.max)
            nc.vector.reciprocal(rr, rr)
            for q in range(WIN // 128):
                col = w * (WIN // 128) + q
                if col >= cfg.ntile:
                    break
                pt = psmall.tile([128, 1], F32, name="pt", tag="ps")
                nc.tensor.transpose(
                    pt[:, :],
                    rrow_sb[:, w * WIN + q * 128:w * WIN + (q + 1) * 128],
                    one11[:, :])
                nc.vector.tensor_copy(rcol_sb[:, col:col + 1], pt[:, :])
            if l == 0:
                for t in range(w * (WIN // 128),
                               min((w + 1) * (WIN // 128), cfg.ntile)):
                    pack_tile(1, t)
                adst_win(1, w)
                if w == cfg.nwin - 1:
                    fire_ag(1)
            else:
                for t in range(w * (WIN // 128),
                               min((w + 1) * (WIN // 128), cfg.ntile)):
                    ph = psmall.tile([128, HS], F32, name="ph", tag="ps")
                    nc.tensor.matmul(ph[:, :],
                                     hT_sb[:, t * 128:(t + 1) * 128],
                                     idn[:, :], start=True, stop=True)
                    hn = packp.tile([128, HS], BF16, name="hn", tag="hn")
                    nc.vector.tensor_scalar(hn[:, :], ph[:, :],
                                            rcol_sb[:, t:t + 1], None,
                                            op0=OP.mult)
                    nc.tensor.matmul(gs_tile[0],
                                     ind_sb[:, t:t + 1, :].squeeze(1),
                                     hn[:, :], start=False,
                                     stop=(t == cfg.ntile - 1))

        def edge_phase(l):
            expand(l)
            win_ps = {}
            for (w, b, lo, n, s0, ks, t_per) in runs:
                if (w, b) not in win_ps:
                    wp = winp.tile([128, WIN], F32, name="wp", tag="wp")
                    win_ps[(w, b)] = wp
                    nc.tensor.matmul(wp[0:NSTA, :], zsta[:, :], zmov[:, :],
                                     start=True, stop=False)
                wp = win_ps[(w, b)]
                ch = chunkp.tile([128, BMAX, TROW], BF16, name="ch", tag="ch")
                gi = chunkp.tile([128, BMAX * 8], I16, name="gi", tag="gi")
                nc.sync.dma_start(out=gi[:, 0:n * 8],
                                  in_=i_gidx[:, lo * 8:(lo + n) * 8])
                tsrc = d_tab[l][b]
                for c0 in range(0, n, GCALL):
                    cn = min(GCALL, n - c0)
                    nc.gpsimd.dma_gather(
                        ch[:, c0:c0 + cn, :],
                        tsrc[:, :],
                        gi[:, c0 * 8:(c0 + cn) * 8],
                        num_idxs=cn * 128, num_idxs_reg=cn * 128,
                        elem_size=TROW)
                # upconvert stationary cols to bf16 on the ACT engine
                chb = chbp.tile([128, BMAX, NSTA], BF16, name="chb",
                                tag="chb")
                nc.scalar.activation(
                    chb[:, 0:n, :],
                    ch[:, 0:n, :].bitcast(FP8)[:, :, 0:NSTA], AF.Copy)
                mk = chunkp.tile([128, BMAX * SUB], BF16, name="mk", tag="mk")
                nc.sync.dma_start(out=mk[:, 0:n * SUB],
                                  in_=i_mask[l][:, lo * SUB:(lo + n) * SUB])
                grid = gridp.tile([128, BMAX, SUB], BF16, name="grid",
                                  tag="grid")
                a0 = w * WIN + s0 * SUB
                nc.vector.tensor_tensor(
                    grid[:, 0:n, :].rearrange("p (s t) j -> p s t j",
                                              t=t_per),
                    ch[:, 0:n, 33:34].squeeze(2)
                        .rearrange("p (s t) -> p s t", t=t_per)
                        .unsqueeze(3)
                        .broadcast_to((128, ks, t_per, SUB)),
                    adst_sb[l][:, a0:a0 + ks * SUB]
                        .rearrange("p (s j) -> p s j", j=SUB)
                        .unsqueeze(2)
                        .broadcast_to((128, ks, t_per, SUB)),
                    op=OP.add)
                nc.vector.tensor_tensor(
                    grid[:, 0:n, :], grid[:, 0:n, :],
                    mk[:, 0:n * SUB].rearrange("p (a j) -> p a j", j=SUB),
                    op=OP.add)
                # exp(lrelu(g)) == max(exp(g), exp(alpha*g))
                oh = ohp.tile([128, BMAX, SUB], BF16, name="oh", tag="oh")
                nc.scalar.activation(oh[:, 0:n, :], grid[:, 0:n, :], AF.Exp)
                e2 = ohp.tile([128, BMAX, SUB], BF16, name="e2", tag="e2")
                nc.scalar.activation(e2[:, 0:n, :], grid[:, 0:n, :], AF.Exp,
                                     scale=ALPHA)
                nc.vector.tensor_tensor(oh[:, 0:n, :], oh[:, 0:n, :],
                                        e2[:, 0:n, :], op=OP.max)
                for kk in range(n):
                    t = lo + kk
                    s = tiles[t][2]
                    off = (s % cfg.spw) * SUB
                    nc.tensor.matmul(
                        wp[0:NSTA, off:off + SUB],
                        chb[:, kk:kk + 1, :].squeeze(1),
                        oh[:, kk:kk + 1, :].squeeze(1),
                        start=False, stop=bool(stop[t]))
                    if act[t] == 1:
                        wps = [win_ps[(w, bb)] for bb in range(cfg.k)
                               if (w, bb) in win_ps]
                        epilogue_final(l, w, wps)

        # ---- main schedule ----
        for t in range(cfg.ntile):
            pack_tile(0, t)
        fire_ag(0)
        for w in range(cfg.nwin):
            adst_win(0, w)
        edge_phase(0)
        gs = gsp.tile([cfg.G, HS], F32, name="gs")
        gs_tile[0] = gs[:, :]
        nc.tensor.matmul(gs[:, :], zsta[:, 0:cfg.G], zmov[:, 0:HS],
                         start=True, stop=False)
        edge_phase(1)
        og = packp.tile([cfg.G, HS], F32, name="og", tag="og")
        nc.vector.tensor_copy(og[:, :], gs[:, :])
        nc.sync.dma_start(out=o_gsum[:, :], in_=og[:, :])

    nc.compile()
    return nc


# ---------------------------------------------------------------------------
# entry point
# ---------------------------------------------------------------------------

def _host_finish(gsums, inputs, cfg):
    batch = np.asarray(inputs["batch"]).astype(np.int64)
    counts = np.bincount(batch, minlength=cfg.G).astype(np.float32)
    total = np.sum(np.stack([np.asarray(g, np.float32) for g in gsums]), 0)
    graph = total / np.maximum(counts[:, None], 1.0)
    gf = np.asarray(inputs["global_features"], np.float32)
    g = gf @ np.asarray(inputs["W_glob"], np.float32) + np.asarray(
        inputs["b_glob"], np.float32)
    comb = np.concatenate([graph, g], 1)
    comb = np.maximum(comb @ np.asarray(inputs["W_comb"], np.float32)
                      + np.asarray(inputs["b_comb"], np.float32), 0.0)
    out = comb @ np.asarray(inputs["W_out"], np.float32) + np.asarray(
        inputs["b_out"], np.float32)
    return out.astype(np.float32)


def run(inputs, cfg, use_sim=False, trace=False):
    in_maps, st = preprocess(inputs, cfg)
    nc = build_program(cfg, st)
    if use_sim:
        from concourse.bass_interp import MultiCoreSim
        # expanded-table cols 68:256 are never written (garbage rides the
        # 256B gather rows, unused downstream) — disable sim NaN guards
        sim = MultiCoreSim(nc, cfg.n_cores, require_finite=False,
                           require_nnan=False)
        for c in range(cfg.n_cores):
            for k, v in in_maps[c].items():
                sim.cores[c].tensor(k)[:] = v
        sim.simulate()
        gsums = [sim.cores[c].mem_tensor("gsum").copy()
                 for c in range(cfg.n_cores)]
        return _host_finish(gsums, inputs, cfg), None
    res = run_bass_kernel_spmd(nc, in_maps, core_ids=list(range(cfg.n_cores)),
                               trace=trace)
    gsums = [res.results[c]["gsum"] for c in range(cfg.n_cores)]
    return _host_finish(gsums, inputs, cfg), res


def kernel(**inputs) -> np.ndarray:
    cfg = Cfg(N=50000, E=1200000, G=25, n_cores=8, F_IN=128)
    out, _ = run(inputs, cfg)
    return out
